# revision 4
# baseline (speedup 1.0000x reference)
"""Trainium2 Bass kernel for nn_Attention_3126736192307.

Causal multi-head attention with RoPE: B=2, S=2048, H=2048, 16 heads x 128.

Sharding (tensor parallel over heads, 8 cores, 2 heads each):
  - Wq/Wk/Wv column-split (per-head), Wo row-split; each core computes a
    partial [B*S, H] output; the host sums the 8 partials (row-parallel
    unshard) - no on-device collectives needed.

Per-core dataflow (all matmuls transpose-free by construction):
  - Host pre-transposes: X.T [H, T], WqT/WkT [H, 256] (head-dim permuted so
    RoPE's rotate_half becomes an intra-quadrant stream_shuffle), WvT [H, 256],
    WoT [256, H], cos/sin [128, T] feature-major (sin sign-folded).
  - Phase 1 per 512-token chunk: three accumulation passes (v token-major,
    then q, then k feature-major [128, T] + RoPE on DVE), each into its own
    2-bank PSUM tag; tags rotate (3c+r) % 4 so every eviction has a full
    ~7.5us pass of slack before its banks are reused -> no PE stalls.
  - Phase 2 per (b, h, i-chunk): scores.T [j,i] = k.T (lhsT) @ q.T; exp on
    ScalarE (no max subtraction - scores are ~N(0,1) after the 1/sqrt(hd)
    scale); causal block skipping + 0/1 mask multiply on diagonal-crossing
    tiles; column sums via ones-matmul on TensorE; AV accumulation in PSUM;
    normalization folded into the PSUM->SBUF eviction.
  - Phase 3: out.T (lhsT) @ WoT -> partial [T, H]; oc-paired 2-bank PSUM
    tiles, evictions alternate DVE/ScalarE, 1KB-row DMA to DRAM.

Matmuls run in bf16 (1 PE cycle/row; fp32 is 4x). LDWEIGHTS is fully hidden
under the matmul stream (verified in trace), so per-MM cost ~= N columns.
"""

import os
import sys

for _p in ("/opt/trn_rl_repo", "/root/.axon_site/_ro/trn_rl_repo"):
    if os.path.isdir(_p) and _p not in sys.path:
        sys.path.append(_p)

from contextlib import ExitStack

import ml_dtypes
import numpy as np

import concourse.bass as bass
import concourse.bacc as bacc
import concourse.tile as tile
from concourse import mybir
from concourse.bass_utils import run_bass_kernel_spmd

B, S, H, NH = 2, 2048, 2048, 16
HD = 128
NCORES = 8
HPC = NH // NCORES            # heads per core = 2
M = HPC * HD                  # 256 output channels per core
SCALE = HD ** -0.5
P = 128                       # partitions
NKT = H // P                  # 16 contraction tiles for projections

F32 = mybir.dt.float32

# head-dim permutation: interleave halves at 16 granularity so the RoPE
# partner (d <-> d+64) sits 16 partitions away inside one 32-part quadrant
PERM = np.concatenate([np.arange(16 * m, 16 * m + 16) + (64 if odd else 0)
                       for m in range(4) for odd in (0, 1)])
SWAP_MASK = [i ^ 16 for i in range(32)]


BF16 = ml_dtypes.bfloat16


def build_masks(tchunk):
    """0/1 keep-masks for the R diagonal-crossing j-tiles of each i-chunk."""
    r = tchunk // P
    m = np.zeros((r, P, tchunk), np.float32)
    il = np.arange(tchunk)
    for ri in range(r):
        for jl in range(P):
            m[ri, jl, :] = (P * ri + jl <= il).astype(np.float32)
    return m


def build_nc(s=S, b=B, tchunk=512, mm_dtype=mybir.dt.bfloat16):
    t = b * s
    tchunk = min(tchunk, t)
    nch = t // tchunk             # phase-1 token chunks
    ich = s // tchunk             # attention i-chunks per batch
    r_mask = tchunk // P          # diagonal-crossing tiles per i-chunk
    ntt = t // P                  # token tiles
    nvp = tchunk // P             # token tiles per chunk

    FR = mm_dtype

    nc = bacc.Bacc("TRN2", target_bir_lowering=False, debug=False)

    xt = nc.declare_dram_parameter("xt", [H, t], FR, isOutput=False)
    wqt = nc.declare_dram_parameter("wqt", [H, M], FR, isOutput=False)
    wkt = nc.declare_dram_parameter("wkt", [H, M], FR, isOutput=False)
    wvt = nc.declare_dram_parameter("wvt", [H, M], FR, isOutput=False)
    wot = nc.declare_dram_parameter("wot", [M, H], FR, isOutput=False)
    cost = nc.declare_dram_parameter("cost", [HD, t], F32, isOutput=False)
    sint = nc.declare_dram_parameter("sint", [HD, t], F32, isOutput=False)
    masks = nc.declare_dram_parameter("masks", [r_mask, P, tchunk], FR,
                                      isOutput=False)
    out = nc.declare_dram_parameter("out", [t, H], FR, isOutput=True)

    with tile.TileContext(nc) as tc, ExitStack() as ctx:
        persist = ctx.enter_context(tc.tile_pool(name="persist", bufs=1))

        # persistent activations
        qr = [persist.tile([P, t], FR, tag=f"qr{h}", name=f"qr{h}") for h in range(HPC)]
        kr = [persist.tile([P, t], FR, tag=f"kr{h}", name=f"kr{h}") for h in range(HPC)]
        vv = persist.tile([P, ntt, M], FR, tag="vv")   # v[tt*128+p, d]
        ones_s = persist.tile([P, P], FR, tag="ones")
        nc.vector.memset(ones_s[:], 1.0)
        warm = persist.tile([P, 1], F32, tag="warm")
        nc.vector.memset(warm[:], 0.0)
        # allocated up-front (fresh SBUF -> no reuse waits on their DMAs);
        # loads issued after phase-1 DMA issues so they don't delay them
        mask_s = persist.tile([P, r_mask, tchunk], FR, tag="masks")
        wo_s = persist.tile([P, HPC, H], FR, tag="wo")
        ev_pool = ctx.enter_context(tc.tile_pool(name="evp", bufs=6))
        # single whole-kernel PSUM pool: four 2-bank tags.  Phase 1 rotates
        # (v, q, k) accumulators through them with a full pass of slack
        # before reuse; phase 2 uses S0/S1 for score tiles, S2 for colsum,
        # S3 for AV; phase 3 pairs oc-outputs on S2/S3.
        pp = ctx.enter_context(tc.tile_pool(name="pp", bufs=1, space="PSUM"))
        STAGS = ["S0", "S1", "S2", "S3"]

        # prime the exp table load + HAM warmup during the initial DMA wait
        wtmp = persist.tile([P, 1], F32, tag="wtmp")
        nc.scalar.activation(out=wtmp[:], in_=warm[:],
                             func=mybir.ActivationFunctionType.Exp)
        warm_ps = pp.tile([P, 2, 512], F32, tag="S3", name="warm_ps")
        for wi in range(40):
            nc.tensor.matmul(warm_ps[:, 0, :P], ones_s[:], ones_s[:],
                             start=True, stop=True)

        # ---------------- phase 1: projections + rope -----------------
        with (
            tc.tile_pool(name="csin", bufs=2) as csin_pool,
            tc.tile_pool(name="xtp", bufs=2) as xt_pool,
            tc.tile_pool(name="rtmp", bufs=3) as rtmp_pool,
            tc.tile_pool(name="wts", bufs=1) as wts_pool,
        ):
            wq_s = wts_pool.tile([P, NKT, M], FR, tag="wq")
            wk_s = wts_pool.tile([P, NKT, M], FR, tag="wk")
            wv_s = wts_pool.tile([P, NKT, M], FR, tag="wv")
            KG = 4                       # k-tiles per DMA
            # weight loads on the (otherwise idle) gpsimd SWDGE queue --
            # wv first (needed by the first v-pass), then wq, wk
            for w_s, wsrc in ((wv_s, wvt), (wq_s, wqt), (wk_s, wkt)):
                for g in range(NKT // KG):
                    gsl = slice(g * KG * P, (g + 1) * KG * P)
                    nc.gpsimd.dma_start(
                        out=w_s[:, g * KG:(g + 1) * KG, :],
                        in_=wsrc[gsl, :].rearrange("(k p) m -> p k m", p=P))

            for c in range(nch):
                tsl = slice(c * tchunk, (c + 1) * tchunk)
                cos_t = csin_pool.tile([P, tchunk], F32, tag="cos")
                sin_t = csin_pool.tile([P, tchunk], F32, tag="sin")
                nc.gpsimd.dma_start(out=cos_t[:], in_=cost[:, tsl])
                nc.gpsimd.dma_start(out=sin_t[:], in_=sint[:, tsl])

                xc = xt_pool.tile([P, NKT, tchunk], FR, tag="xt")
                if c == 0:
                    # split the first group so the first v-matmul can start
                    # as soon as one k-tile has landed
                    nc.sync.dma_start(
                        out=xc[:, 0:1, :],
                        in_=xt[0:P, tsl].rearrange("(k p) t -> p k t", p=P))
                    nc.sync.dma_start(
                        out=xc[:, 1:KG, :],
                        in_=xt[P:KG * P, tsl].rearrange("(k p) t -> p k t",
                                                        p=P))
                    ggs = range(1, NKT // KG)
                else:
                    ggs = range(NKT // KG)
                for g in ggs:
                    gsl = slice(g * KG * P, (g + 1) * KG * P)
                    nc.sync.dma_start(
                        out=xc[:, g * KG:(g + 1) * KG, :],
                        in_=xt[gsl, tsl].rearrange("(k p) t -> p k t", p=P))

                vtag = STAGS[(3 * c) % 4]
                qtag = STAGS[(3 * c + 1) % 4]
                ktag = STAGS[(3 * c + 2) % 4]

                # v pass: token-major [tok, d], x tiles stationary
                v_ps = pp.tile([P, nvp, M], F32, tag=vtag, name=f"vps_{c}")
                for kt in range(NKT):
                    for ts_ in range(nvp):
                        ssl = slice(ts_ * P, (ts_ + 1) * P)
                        # start only on the first group touching each 2KB
                        # bank: start marks the WHOLE bank pending-zero, so
                        # the bank-sharing sibling group must rely on the
                        # cleared has_written bits to overwrite at kt=0
                        fl = dict(start=(kt == 0 and ts_ % 2 == 0),
                                  stop=(kt == NKT - 1))
                        nc.tensor.matmul(v_ps[:, ts_, :], xc[:, kt, ssl],
                                         wv_s[:, kt, :], **fl)
                nc.vector.tensor_copy(out=vv[:, c * nvp:(c + 1) * nvp, :],
                                      in_=v_ps[:])

                # q pass, then k pass: feature-major [d, tok] + rope
                for wname, w_s, dest, ptag in (("q", wq_s, qr, qtag),
                                               ("k", wk_s, kr, ktag)):
                    ps = pp.tile([P, HPC, tchunk], F32, tag=ptag,
                                 name=f"{wname}ps_{c}")
                    for kt in range(NKT):
                        fl = dict(start=(kt == 0), stop=(kt == NKT - 1))
                        for h in range(HPC):
                            msl = slice(h * P, (h + 1) * P)
                            nc.tensor.matmul(ps[:, h, :], w_s[:, kt, msl],
                                             xc[:, kt, :], **fl)
                    # rope eviction: dest = ps*cos + shuffle(ps)*sin_eff
                    for h in range(HPC):
                        shuf = rtmp_pool.tile([P, tchunk], F32, tag="shuf")
                        dst = dest[h][:, tsl]
                        nc.vector.stream_shuffle(out=shuf[:], in_=ps[:, h, :],
                                                 mask=SWAP_MASK)
                        nc.vector.tensor_mul(out=dst, in0=ps[:, h, :],
                                             in1=cos_t[:])
                        nc.vector.tensor_mul(out=shuf[:], in0=shuf[:],
                                             in1=sin_t[:])
                        nc.vector.tensor_add(out=dst, in0=dst, in1=shuf[:])

        nc.sync.dma_start(out=mask_s[:], in_=masks.rearrange("r p n -> p r n"))
        nc.sync.dma_start(out=wo_s[:],
                          in_=wot.rearrange("(mt p) o -> p mt o", p=P))

        # -------- phase 2+3: attention with interleaved output proj -------
        # Software-pipelined: QK for tile jt+1 issues before colsum/AV of jt,
        # and both heads' exp runs as ONE wide ACT op over a 2-bank PSUM
        # tile, so ACT latency never blocks the PE stream.
        with (
            tc.tile_pool(name="outp", bufs=1) as out_pool,
            tc.tile_pool(name="exps", bufs=8) as exps_pool,
            tc.tile_pool(name="rcp", bufs=2) as rcp_pool,
        ):
            outT = [out_pool.tile([P, t], FR, tag=f"outT{h}", name=f"outT{h}")
                    for h in range(HPC)]

            def drain_one(pend):
                (pes, plo, pw, pfl, pjt, ctx_) = pend.pop(0)
                (bb_, cs_l, av_l, isl_, c_) = ctx_
                for h in range(HPC):
                    nc.tensor.matmul(cs_l[:, h, plo:], ones_s[:],
                                     pes[:, h, :pw], **pfl)
                    nc.tensor.matmul(av_l[:, h, plo:],
                                     vv[:, bb_ * (s // P) + pjt,
                                        h * P:(h + 1) * P],
                                     pes[:, h, :pw], **pfl)
                if not pfl["stop"]:
                    return
                # chunk epilogue: normalize + output projection
                for h in range(HPC):
                    rcp = rcp_pool.tile([P, tchunk], F32, tag="rcp",
                                        name=f"rcp{h}_{bb_}_{c_}")
                    nc.vector.reciprocal_approx_fast(out=rcp[:],
                                                     in_=cs_l[:, h, :])
                    nc.vector.tensor_mul(out=outT[h][:, isl_],
                                         in0=av_l[:, h, :], in1=rcp[:])
                wi_ = 0
                for tt_ in range(tchunk // P):
                    tt0 = isl_.start + tt_ * P
                    ttsl = slice(tt0, tt0 + P)
                    for op_ in range(H // 1024):
                        ps = pp.tile([P, 2, 512], F32,
                                     tag=STAGS[2 + (wi_ % 2)],
                                     name=f"wo_{tt0}_{op_}")
                        for sub in range(2):
                            osl = slice((op_ * 2 + sub) * 512,
                                        (op_ * 2 + sub + 1) * 512)
                            for h in range(HPC):
                                nc.tensor.matmul(ps[:, sub, :],
                                                 outT[h][:, ttsl],
                                                 wo_s[:, h, osl],
                                                 start=(h == 0),
                                                 stop=(h == HPC - 1))
                        ev = ev_pool.tile([P, 1024], FR, tag="ev",
                                          name=f"ev_{tt0}_{op_}")
                        # alternate eviction between DVE and ScalarE
                        if wi_ % 2 == 0:
                            nc.vector.tensor_copy(
                                out=ev[:], in_=ps.rearrange("p a b -> p (a b)"))
                        else:
                            nc.scalar.copy(
                                out=ev[:], in_=ps.rearrange("p a b -> p (a b)"))
                        nc.sync.dma_start(
                            out=out[ttsl, op_ * 1024:(op_ + 1) * 1024],
                            in_=ev[:])
                        wi_ += 1

            pend = []
            for bb in range(b):
                for c in range(ich):
                    isl = slice(bb * s + c * tchunk, bb * s + (c + 1) * tchunk)
                    njt = r_mask * (c + 1)   # visible j-tiles
                    cs_ps = pp.tile([P, HPC, tchunk], F32, tag="S2",
                                    name=f"cs_{bb}_{c}")
                    av_ps = pp.tile([P, HPC, tchunk], F32, tag="S3",
                                    name=f"av_{bb}_{c}")
                    cctx = (bb, cs_ps, av_ps, isl, c)
                    for jt in range(njt):
                        jsl = slice(bb * s + jt * P, bb * s + (jt + 1) * P)
                        ri = jt - r_mask * c
                        lo = max(ri, 0) * P
                        w = tchunk - lo
                        csl = slice(isl.start + lo, isl.stop)
                        fl = dict(start=(jt == 0), stop=(jt == njt - 1))
                        sc = pp.tile([P, HPC, tchunk], F32,
                                     tag=STAGS[jt % 2],
                                     name=f"sc_{bb}_{c}_{jt}")
                        for h in range(HPC):
                            nc.tensor.matmul(sc[:, h, :w], kr[h][:, jsl],
                                             qr[h][:, csl],
                                             start=True, stop=True)
                        es = exps_pool.tile([P, HPC, tchunk], FR, tag="es",
                                            name=f"es_{bb}_{c}_{jt}")
                        nc.scalar.activation(out=es[:, :, :w], in_=sc[:, :, :w],
                                             func=mybir.ActivationFunctionType.Exp,
                                             scale=float(SCALE))
                        if ri >= 0:  # diagonal-crossing tile
                            mb = mask_s[:, ri, lo:].unsqueeze(1).broadcast_to(
                                [P, HPC, w])
                            nc.vector.tensor_mul(out=es[:, :, :w],
                                                 in0=es[:, :, :w], in1=mb)
                        pend.append((es, lo, w, fl, jt, cctx))
                        if len(pend) > 2:
                            drain_one(pend)
            while pend:
                drain_one(pend)

    nc.compile()
    return nc


def make_in_maps(hidden_states, cos, sin, Wq, Wk, Wv, Wo, s=S, b=B, tchunk=512):
    t = b * s
    tchunk = min(tchunk, t)
    hs = np.asarray(hidden_states, np.float32).reshape(t, H)
    xt = np.ascontiguousarray(hs.T)
    cos2 = np.asarray(cos, np.float32).reshape(s, HD)
    sin2 = np.asarray(sin, np.float32).reshape(s, HD)
    cosP = np.ascontiguousarray(np.tile(cos2[:, PERM].T, (1, b)))
    sign = np.where(PERM < 64, -1.0, 1.0).astype(np.float32)[:, None]
    sinP = np.ascontiguousarray(np.tile(sin2[:, PERM].T * sign, (1, b)))
    masks_bf = build_masks(tchunk).astype(BF16)
    xt_bf = xt.astype(BF16)
    Wq, Wk, Wv, Wo = (np.asarray(w, np.float32) for w in (Wq, Wk, Wv, Wo))

    in_maps = []
    for c in range(NCORES):
        rows = np.concatenate([(HPC * c + hh) * HD + PERM for hh in range(HPC)])
        sl = slice(c * M, (c + 1) * M)
        in_maps.append({
            "xt": xt_bf,
            "wqt": np.ascontiguousarray(Wq[rows, :].T).astype(BF16),
            "wkt": np.ascontiguousarray(Wk[rows, :].T).astype(BF16),
            "wvt": np.ascontiguousarray(Wv[sl, :].T).astype(BF16),
            "wot": np.ascontiguousarray(Wo[:, sl].T).astype(BF16),
            "cost": cosP,
            "sint": sinP,
            "masks": masks_bf,
        })
    return in_maps


_CACHED_NC = None
_LAST_RESULTS = None


def kernel(hidden_states, cos, sin, Wq, Wk, Wv, Wo):
    global _CACHED_NC, _LAST_RESULTS
    in_maps = make_in_maps(hidden_states, cos, sin, Wq, Wk, Wv, Wo)
    if _CACHED_NC is None:
        _CACHED_NC = build_nc()
    res = run_bass_kernel_spmd(_CACHED_NC, in_maps, core_ids=list(range(NCORES)))
    _LAST_RESULTS = res
    acc = np.zeros((B * S, H), np.float32)
    for r in res.results:
        acc += r["out"].astype(np.float32)
    return acc.reshape(B, S, H)


# revision 8
# speedup vs baseline: 1.0053x; 1.0053x over previous
"""Trainium2 Bass kernel for nn_Attention_3126736192307.

Causal multi-head attention with RoPE: B=2, S=2048, H=2048, 16 heads x 128.

Sharding (tensor parallel over heads, 8 cores, 2 heads each):
  - Wq/Wk/Wv column-split (per-head), Wo row-split; each core computes a
    partial [B*S, H] output; the host sums the 8 partials (row-parallel
    unshard) - no on-device collectives needed.

Per-core dataflow (all matmuls transpose-free by construction):
  - Host pre-transposes: X.T [H, T], WqT/WkT [H, 256] (head-dim permuted so
    RoPE's rotate_half becomes an intra-quadrant stream_shuffle), WvT [H, 256],
    WoT [256, H], cos/sin [128, T] feature-major (sin sign-folded).
  - Phase 1 per 512-token chunk: three accumulation passes (v token-major,
    then q, then k feature-major [128, T] + RoPE on DVE), each into its own
    2-bank PSUM tag; tags rotate (3c+r) % 4 so every eviction has a full
    ~7.5us pass of slack before its banks are reused -> no PE stalls.
  - Phase 2 per (b, h, i-chunk): scores.T [j,i] = k.T (lhsT) @ q.T; exp on
    ScalarE (no max subtraction - scores are ~N(0,1) after the 1/sqrt(hd)
    scale); causal block skipping + 0/1 mask multiply on diagonal-crossing
    tiles; column sums via ones-matmul on TensorE; AV accumulation in PSUM;
    normalization folded into the PSUM->SBUF eviction.
  - Phase 3: out.T (lhsT) @ WoT -> partial [T, H]; oc-paired 2-bank PSUM
    tiles, evictions alternate DVE/ScalarE, 1KB-row DMA to DRAM.

Matmuls run in bf16 (1 PE cycle/row; fp32 is 4x). LDWEIGHTS is fully hidden
under the matmul stream (verified in trace), so per-MM cost ~= N columns.
"""

import os
import sys

for _p in ("/opt/trn_rl_repo", "/root/.axon_site/_ro/trn_rl_repo"):
    if os.path.isdir(_p) and _p not in sys.path:
        sys.path.append(_p)

from contextlib import ExitStack

import ml_dtypes
import numpy as np

import concourse.bass as bass
import concourse.bacc as bacc
import concourse.tile as tile
from concourse import mybir
from concourse.bass_utils import run_bass_kernel_spmd

B, S, H, NH = 2, 2048, 2048, 16
HD = 128
NCORES = 8
HPC = NH // NCORES            # heads per core = 2
M = HPC * HD                  # 256 output channels per core
SCALE = HD ** -0.5
P = 128                       # partitions
NKT = H // P                  # 16 contraction tiles for projections

F32 = mybir.dt.float32

# head-dim permutation: interleave halves at 16 granularity so the RoPE
# partner (d <-> d+64) sits 16 partitions away inside one 32-part quadrant
PERM = np.concatenate([np.arange(16 * m, 16 * m + 16) + (64 if odd else 0)
                       for m in range(4) for odd in (0, 1)])
SWAP_MASK = [i ^ 16 for i in range(32)]


BF16 = ml_dtypes.bfloat16


def build_masks(tchunk):
    """0/1 keep-masks for the R diagonal-crossing j-tiles of each i-chunk."""
    r = tchunk // P
    m = np.zeros((r, P, tchunk), np.float32)
    il = np.arange(tchunk)
    for ri in range(r):
        for jl in range(P):
            m[ri, jl, :] = (P * ri + jl <= il).astype(np.float32)
    return m


def build_nc(s=S, b=B, tchunk=512, mm_dtype=mybir.dt.bfloat16):
    t = b * s
    tchunk = min(tchunk, t)
    nch = t // tchunk             # phase-1 token chunks
    ich = s // tchunk             # attention i-chunks per batch
    r_mask = tchunk // P          # diagonal-crossing tiles per i-chunk
    ntt = t // P                  # token tiles
    nvp = tchunk // P             # token tiles per chunk

    FR = mm_dtype

    nc = bacc.Bacc("TRN2", target_bir_lowering=False, debug=False)

    xt = nc.declare_dram_parameter("xt", [H, t], FR, isOutput=False)
    wqt = nc.declare_dram_parameter("wqt", [H, M], FR, isOutput=False)
    wkt = nc.declare_dram_parameter("wkt", [H, M], FR, isOutput=False)
    wvt = nc.declare_dram_parameter("wvt", [H, M], FR, isOutput=False)
    wot = nc.declare_dram_parameter("wot", [M, H], FR, isOutput=False)
    cost = nc.declare_dram_parameter("cost", [HD, t], F32, isOutput=False)
    sint = nc.declare_dram_parameter("sint", [HD, t], F32, isOutput=False)
    masks = nc.declare_dram_parameter("masks", [r_mask, P, tchunk], FR,
                                      isOutput=False)
    out = nc.declare_dram_parameter("out", [t, H], FR, isOutput=True)

    with tile.TileContext(nc) as tc, ExitStack() as ctx:
        persist = ctx.enter_context(tc.tile_pool(name="persist", bufs=1))

        # persistent activations
        qr = [persist.tile([P, t], FR, tag=f"qr{h}", name=f"qr{h}") for h in range(HPC)]
        kr = [persist.tile([P, t], FR, tag=f"kr{h}", name=f"kr{h}") for h in range(HPC)]
        vv = persist.tile([P, ntt, M], FR, tag="vv")   # v[tt*128+p, d]
        ones_s = persist.tile([P, P], FR, tag="ones")
        nc.vector.memset(ones_s[:], 1.0)
        warm = persist.tile([P, 1], F32, tag="warm")
        nc.vector.memset(warm[:], 0.0)
        # allocated up-front (fresh SBUF -> no reuse waits on their DMAs);
        # loads issued after phase-1 DMA issues so they don't delay them
        mask_s = persist.tile([P, r_mask, tchunk], FR, tag="masks")
        wo_s = persist.tile([P, HPC, H], FR, tag="wo")
        ev_pool = ctx.enter_context(tc.tile_pool(name="evp", bufs=6))
        # single whole-kernel PSUM pool: four 2-bank tags.  Phase 1 rotates
        # (v, q, k) accumulators through them with a full pass of slack
        # before reuse; phase 2 uses S0/S1 for score tiles, S2 for colsum,
        # S3 for AV; phase 3 pairs oc-outputs on S2/S3.
        pp = ctx.enter_context(tc.tile_pool(name="pp", bufs=1, space="PSUM"))
        STAGS = ["S0", "S1", "S2", "S3"]

        # prime the exp table load + HAM warmup during the initial DMA wait
        wtmp = persist.tile([P, 1], F32, tag="wtmp")
        nc.scalar.activation(out=wtmp[:], in_=warm[:],
                             func=mybir.ActivationFunctionType.Exp)
        warm_ps = pp.tile([P, 2, 512], F32, tag="S3", name="warm_ps")
        for wi in range(40):
            nc.tensor.matmul(warm_ps[:, 0, :P], ones_s[:], ones_s[:],
                             start=True, stop=True)

        # ---------------- phase 1: projections + rope -----------------
        with (
            tc.tile_pool(name="csin", bufs=2) as csin_pool,
            tc.tile_pool(name="xtp", bufs=2) as xt_pool,
            tc.tile_pool(name="rtmp", bufs=3) as rtmp_pool,
            tc.tile_pool(name="wts", bufs=1) as wts_pool,
        ):
            wq_s = wts_pool.tile([P, NKT, M], FR, tag="wq")
            wk_s = wts_pool.tile([P, NKT, M], FR, tag="wk")
            wv_s = wts_pool.tile([P, NKT, M], FR, tag="wv")
            KG = 4                       # k-tiles per DMA
            # weight loads on the sync HWDGE queue (transfers parallelize
            # across the 16 DMA engines; the gpsimd SWDGE path serializes
            # and starved the first q/k passes).  wv in 4 groups so the
            # first v-matmul starts early; wq/wk as single transfers.
            for g in range(NKT // KG):
                gsl = slice(g * KG * P, (g + 1) * KG * P)
                nc.sync.dma_start(
                    out=wv_s[:, g * KG:(g + 1) * KG, :],
                    in_=wvt[gsl, :].rearrange("(k p) m -> p k m", p=P))
            for w_s, wsrc in ((wq_s, wqt), (wk_s, wkt)):
                nc.sync.dma_start(
                    out=w_s[:], in_=wsrc.rearrange("(k p) m -> p k m", p=P))

            for c in range(nch):
                tsl = slice(c * tchunk, (c + 1) * tchunk)
                cos_t = csin_pool.tile([P, tchunk], F32, tag="cos")
                sin_t = csin_pool.tile([P, tchunk], F32, tag="sin")
                nc.gpsimd.dma_start(out=cos_t[:], in_=cost[:, tsl])
                nc.gpsimd.dma_start(out=sin_t[:], in_=sint[:, tsl])

                xc = xt_pool.tile([P, NKT, tchunk], FR, tag="xt")
                if c == 0:
                    # split the first group so the first v-matmul can start
                    # as soon as one k-tile has landed
                    nc.sync.dma_start(
                        out=xc[:, 0:1, :],
                        in_=xt[0:P, tsl].rearrange("(k p) t -> p k t", p=P))
                    nc.sync.dma_start(
                        out=xc[:, 1:KG, :],
                        in_=xt[P:KG * P, tsl].rearrange("(k p) t -> p k t",
                                                        p=P))
                    ggs = range(1, NKT // KG)
                else:
                    ggs = range(NKT // KG)
                for g in ggs:
                    gsl = slice(g * KG * P, (g + 1) * KG * P)
                    nc.sync.dma_start(
                        out=xc[:, g * KG:(g + 1) * KG, :],
                        in_=xt[gsl, tsl].rearrange("(k p) t -> p k t", p=P))

                vtag = STAGS[(3 * c) % 4]
                qtag = STAGS[(3 * c + 1) % 4]
                ktag = STAGS[(3 * c + 2) % 4]

                # v pass: token-major [tok, d], x tiles stationary
                v_ps = pp.tile([P, nvp, M], F32, tag=vtag, name=f"vps_{c}")
                for kt in range(NKT):
                    for ts_ in range(nvp):
                        ssl = slice(ts_ * P, (ts_ + 1) * P)
                        # start only on the first group touching each 2KB
                        # bank: start marks the WHOLE bank pending-zero, so
                        # the bank-sharing sibling group must rely on the
                        # cleared has_written bits to overwrite at kt=0
                        fl = dict(start=(kt == 0 and ts_ % 2 == 0),
                                  stop=(kt == NKT - 1))
                        nc.tensor.matmul(v_ps[:, ts_, :], xc[:, kt, ssl],
                                         wv_s[:, kt, :], **fl)
                nc.vector.tensor_copy(out=vv[:, c * nvp:(c + 1) * nvp, :],
                                      in_=v_ps[:])

                # q pass, then k pass: feature-major [d, tok] + rope
                for wname, w_s, dest, ptag in (("q", wq_s, qr, qtag),
                                               ("k", wk_s, kr, ktag)):
                    ps = pp.tile([P, HPC, tchunk], F32, tag=ptag,
                                 name=f"{wname}ps_{c}")
                    for kt in range(NKT):
                        fl = dict(start=(kt == 0), stop=(kt == NKT - 1))
                        for h in range(HPC):
                            msl = slice(h * P, (h + 1) * P)
                            nc.tensor.matmul(ps[:, h, :], w_s[:, kt, msl],
                                             xc[:, kt, :], **fl)
                    # rope eviction: dest = ps*cos + shuffle(ps)*sin_eff
                    for h in range(HPC):
                        shuf = rtmp_pool.tile([P, tchunk], F32, tag="shuf")
                        dst = dest[h][:, tsl]
                        nc.vector.stream_shuffle(out=shuf[:], in_=ps[:, h, :],
                                                 mask=SWAP_MASK)
                        nc.vector.tensor_mul(out=dst, in0=ps[:, h, :],
                                             in1=cos_t[:])
                        nc.vector.tensor_mul(out=shuf[:], in0=shuf[:],
                                             in1=sin_t[:])
                        nc.vector.tensor_add(out=dst, in0=dst, in1=shuf[:])

        nc.sync.dma_start(out=mask_s[:], in_=masks.rearrange("r p n -> p r n"))
        nc.sync.dma_start(out=wo_s[:],
                          in_=wot.rearrange("(mt p) o -> p mt o", p=P))

        # -------- phase 2+3: attention with interleaved output proj -------
        # Software-pipelined: QK for tile jt+1 issues before colsum/AV of jt,
        # and both heads' exp runs as ONE wide ACT op over a 2-bank PSUM
        # tile, so ACT latency never blocks the PE stream.
        with (
            tc.tile_pool(name="outp", bufs=1) as out_pool,
            tc.tile_pool(name="exps", bufs=8) as exps_pool,
            tc.tile_pool(name="rcp", bufs=2) as rcp_pool,
        ):
            outT = [out_pool.tile([P, t], FR, tag=f"outT{h}", name=f"outT{h}")
                    for h in range(HPC)]

            def drain_one(pend):
                (pes, plo, pw, pfl, pjt, ctx_) = pend.pop(0)
                (bb_, cs_l, av_l, isl_, c_) = ctx_
                if not pfl["stop"]:
                    for h in range(HPC):
                        nc.tensor.matmul(cs_l[:, h, plo:], ones_s[:],
                                         pes[:, h, :pw], **pfl)
                        nc.tensor.matmul(av_l[:, h, plo:],
                                         vv[:, bb_ * (s // P) + pjt,
                                            h * P:(h + 1) * P],
                                         pes[:, h, :pw], **pfl)
                    return
                # last tile: colsums first so the reciprocals (DVE) overlap
                # the trailing AV matmuls instead of serializing after them
                rcps = []
                for h in range(HPC):
                    nc.tensor.matmul(cs_l[:, h, plo:], ones_s[:],
                                     pes[:, h, :pw], **pfl)
                for h in range(HPC):
                    rcp = rcp_pool.tile([P, tchunk], F32, tag="rcp",
                                        name=f"rcp{h}_{bb_}_{c_}")
                    nc.vector.reciprocal_approx_fast(out=rcp[:],
                                                     in_=cs_l[:, h, :])
                    rcps.append(rcp)
                for h in range(HPC):
                    nc.tensor.matmul(av_l[:, h, plo:],
                                     vv[:, bb_ * (s // P) + pjt,
                                        h * P:(h + 1) * P],
                                     pes[:, h, :pw], **pfl)
                for h in range(HPC):
                    nc.vector.tensor_mul(out=outT[h][:, isl_],
                                         in0=av_l[:, h, :], in1=rcps[h][:])
                wi_ = 0
                for tt_ in range(tchunk // P):
                    tt0 = isl_.start + tt_ * P
                    ttsl = slice(tt0, tt0 + P)
                    for op_ in range(H // 1024):
                        ps = pp.tile([P, 2, 512], F32,
                                     tag=STAGS[2 + (wi_ % 2)],
                                     name=f"wo_{tt0}_{op_}")
                        for sub in range(2):
                            osl = slice((op_ * 2 + sub) * 512,
                                        (op_ * 2 + sub + 1) * 512)
                            for h in range(HPC):
                                nc.tensor.matmul(ps[:, sub, :],
                                                 outT[h][:, ttsl],
                                                 wo_s[:, h, osl],
                                                 start=(h == 0),
                                                 stop=(h == HPC - 1))
                        ev = ev_pool.tile([P, 1024], FR, tag="ev",
                                          name=f"ev_{tt0}_{op_}")
                        # DVE-only eviction: ScalarE copies here congest the
                        # exp critical path (ACT queue is strict FIFO)
                        nc.vector.tensor_copy(
                            out=ev[:], in_=ps.rearrange("p a b -> p (a b)"))
                        nc.sync.dma_start(
                            out=out[ttsl, op_ * 1024:(op_ + 1) * 1024],
                            in_=ev[:])
                        wi_ += 1

            pend = []
            for bb in range(b):
                for c in range(ich):
                    isl = slice(bb * s + c * tchunk, bb * s + (c + 1) * tchunk)
                    njt = r_mask * (c + 1)   # visible j-tiles
                    cs_ps = pp.tile([P, HPC, tchunk], F32, tag="S2",
                                    name=f"cs_{bb}_{c}")
                    av_ps = pp.tile([P, HPC, tchunk], F32, tag="S3",
                                    name=f"av_{bb}_{c}")
                    cctx = (bb, cs_ps, av_ps, isl, c)
                    for jt in range(njt):
                        jsl = slice(bb * s + jt * P, bb * s + (jt + 1) * P)
                        ri = jt - r_mask * c
                        lo = max(ri, 0) * P
                        w = tchunk - lo
                        csl = slice(isl.start + lo, isl.stop)
                        fl = dict(start=(jt == 0), stop=(jt == njt - 1))
                        sc = pp.tile([P, HPC, tchunk], F32,
                                     tag=STAGS[jt % 2],
                                     name=f"sc_{bb}_{c}_{jt}")
                        for h in range(HPC):
                            nc.tensor.matmul(sc[:, h, :w], kr[h][:, jsl],
                                             qr[h][:, csl],
                                             start=True, stop=True)
                        es = exps_pool.tile([P, HPC, tchunk], FR, tag="es",
                                            name=f"es_{bb}_{c}_{jt}")
                        nc.scalar.activation(out=es[:, :, :w], in_=sc[:, :, :w],
                                             func=mybir.ActivationFunctionType.Exp,
                                             scale=float(SCALE))
                        if ri >= 0:  # diagonal-crossing tile
                            mb = mask_s[:, ri, lo:].unsqueeze(1).broadcast_to(
                                [P, HPC, w])
                            nc.vector.tensor_mul(out=es[:, :, :w],
                                                 in0=es[:, :, :w], in1=mb)
                        pend.append((es, lo, w, fl, jt, cctx))
                        if len(pend) > 3:
                            drain_one(pend)
            while pend:
                drain_one(pend)

    nc.compile()
    return nc


def make_in_maps(hidden_states, cos, sin, Wq, Wk, Wv, Wo, s=S, b=B, tchunk=512):
    t = b * s
    tchunk = min(tchunk, t)
    hs = np.asarray(hidden_states, np.float32).reshape(t, H)
    xt = np.ascontiguousarray(hs.T)
    cos2 = np.asarray(cos, np.float32).reshape(s, HD)
    sin2 = np.asarray(sin, np.float32).reshape(s, HD)
    cosP = np.ascontiguousarray(np.tile(cos2[:, PERM].T, (1, b)))
    sign = np.where(PERM < 64, -1.0, 1.0).astype(np.float32)[:, None]
    sinP = np.ascontiguousarray(np.tile(sin2[:, PERM].T * sign, (1, b)))
    masks_bf = build_masks(tchunk).astype(BF16)
    xt_bf = xt.astype(BF16)
    Wq, Wk, Wv, Wo = (np.asarray(w, np.float32) for w in (Wq, Wk, Wv, Wo))

    in_maps = []
    for c in range(NCORES):
        rows = np.concatenate([(HPC * c + hh) * HD + PERM for hh in range(HPC)])
        sl = slice(c * M, (c + 1) * M)
        in_maps.append({
            "xt": xt_bf,
            "wqt": np.ascontiguousarray(Wq[rows, :].T).astype(BF16),
            "wkt": np.ascontiguousarray(Wk[rows, :].T).astype(BF16),
            "wvt": np.ascontiguousarray(Wv[sl, :].T).astype(BF16),
            "wot": np.ascontiguousarray(Wo[:, sl].T).astype(BF16),
            "cost": cosP,
            "sint": sinP,
            "masks": masks_bf,
        })
    return in_maps


_CACHED_NC = None
_LAST_RESULTS = None


def kernel(hidden_states, cos, sin, Wq, Wk, Wv, Wo):
    global _CACHED_NC, _LAST_RESULTS
    in_maps = make_in_maps(hidden_states, cos, sin, Wq, Wk, Wv, Wo)
    if _CACHED_NC is None:
        _CACHED_NC = build_nc()
    res = run_bass_kernel_spmd(_CACHED_NC, in_maps, core_ids=list(range(NCORES)))
    _LAST_RESULTS = res
    acc = np.zeros((B * S, H), np.float32)
    for r in res.results:
        acc += r["out"].astype(np.float32)
    return acc.reshape(B, S, H)
